# revision 5
# baseline (speedup 1.0000x reference)
"""Trainium2 Bass kernel for nn_BasicCGInducer (compound-PCFG grammar inducer).

Reference computation (see problem spec):
    full_p0      = log_softmax(root_mask + root_w + root_b)            [2048]
    rule_scores  = log_softmax(rule_w + rule_b, axis=1)                [1024, 2048]
    h            = 2x ResidualLayer(nt_emb @ sw1 + sb1)               [2048, 1024]
    split_scores = log_softmax(h @ sow + sob, axis=1)                  [2048, 2]
    full_G_larg  = rule_scores[:, :1024] + split_scores[:1024, 0:1]
    full_G_rarg  = rule_scores[:, 1024:] + split_scores[:1024, 0:1]
    word_lp      = log_softmax(nt_emb @ emit_w + emit_b, axis=1)       [2048, 32000]
    x_scores     = word_lp[:, x].transpose(1, 2, 0)                    [32, 128, 2048]

Sharding over 8 cores:
  * emission GEMM + logsumexp stats: vocab-sharded (4000 cols/core), fused
    streaming logsumexp (running row-max on DVE, exp+row-sum fused on ACT).
  * x_scores: token-sharded (512 of 4096 tokens/core); emit_w columns are
    host-gathered; the kernel emits raw gathered logits (+emit_b); host
    combines per-core (max, sumexp) stats into the global logsumexp and
    subtracts.
  * grammar MLP: q-sharded 256 cols/core in transposed space, aligned so that
    core c owns split rows [128c,128c+128) of the first 1024 categories and
    the matching 128-row shard of rule_w, making G_larg/G_rarg fully local.
  * all GEMMs run in float32r (full PE rate; ~1e-4 rel err vs fp32).
"""

import sys

for _p in ("/opt/trn_rl_repo", "/opt/pypackages"):
    if _p not in sys.path:
        sys.path.append(_p)

import numpy as np

import concourse.bass as bass
import concourse.tile as tile
from concourse import bacc, mybir
from concourse.bass_utils import run_bass_kernel_spmd

F32 = mybir.dt.float32
F32R = mybir.dt.float32r  # fp32 bits, fast PE path
GEMM_DT = F32R

NCORE = 8
D, V, B, S = 1024, 32000, 32, 128
QA, QR = 2048, 1024
T = B * S            # 4096 tokens
TS = T // NCORE      # 512 tokens per core
VS = V // NCORE      # 4000 vocab cols per core
KT = D // 128        # 8 contraction k-tiles
QB = QA // 128       # 16 q row-blocks
CH = 500             # vocab chunk width per stats iteration
NCH = VS // CH       # 8 chunks
QS = 256             # per-core MLP q columns (2 slices of 128)

AF = mybir.ActivationFunctionType
AX = mybir.AxisListType
OP = mybir.AluOpType


def _emit(tc, A):
    nc = tc.nc

    import contextlib
    ctx = contextlib.ExitStack()
    with ctx:
        # ---------- persistent pools ----------
        x0p = ctx.enter_context(tc.tile_pool(name="x0p", bufs=1))
        inp = ctx.enter_context(tc.tile_pool(name="inp", bufs=1))
        actp = ctx.enter_context(tc.tile_pool(name="actp", bufs=3))
        wpool = ctx.enter_context(tc.tile_pool(name="wpool", bufs=2))
        smallp = ctx.enter_context(tc.tile_pool(name="smallp", bufs=4))
        statp = ctx.enter_context(tc.tile_pool(name="statp", bufs=1))
        outp = ctx.enter_context(tc.tile_pool(name="outp", bufs=1))
        splitp = ctx.enter_context(tc.tile_pool(name="splitp", bufs=1))

        # X0 = nt_emb^T, laid out [p, kt*2048 + q]
        x0 = x0p.tile([128, KT * QA], GEMM_DT)
        nc.sync.dma_start(
            x0[:].rearrange("p (a n) -> p a n", a=KT),
            A["x0T"].rearrange("(a p) n -> p a n", p=128),
        )
        # MLP input H0 = nt_emb[qsel]^T, [p, kt*256 + q]
        h0 = inp.tile([128, KT * QS], GEMM_DT)
        nc.sync.dma_start(
            h0[:].rearrange("p (a n) -> p a n", a=KT),
            A["h0T"].rearrange("(a p) n -> p a n", p=128),
        )
        # rule_w shard + rule_b (host pre-added)
        rwb = inp.tile([128, QA], F32)
        nc.sync.dma_start(rwb[:], A["rwb"][:])
        # per-dout-block bias tiles [p, dout_block]
        biases = {}
        for nm in ("b_s", "b_r11", "b_r12", "b_r21", "b_r22"):
            bt = inp.tile([128, KT], F32, tag=nm)
            nc.sync.dma_start(bt[:], A[nm].rearrange("(a p) -> p a", p=128))
            biases[nm] = bt
        sow_t = inp.tile([128, KT * 2], GEMM_DT)
        nc.sync.dma_start(
            sow_t[:].rearrange("p (a n) -> p a n", a=KT),
            A["sow"].rearrange("(a p) n -> p a n", p=128),
        )
        sob_a = inp.tile([1, 1], F32, tag="sob_a")
        sob_b = inp.tile([1, 1], F32, tag="sob_b")
        nc.sync.dma_start(sob_a[:], A["sob"].rearrange("(a n) -> a n", a=1)[:, 0:1])
        nc.sync.dma_start(sob_b[:], A["sob"].rearrange("(a n) -> a n", a=1)[:, 1:2])
        ebg_t = inp.tile([128, 4], F32)
        nc.sync.dma_start(ebg_t[:], A["ebg"].rearrange("(a p) -> p a", p=128))
        ones = inp.tile([1, 128], GEMM_DT)
        nc.sync.dma_start(ones[:], A["ones_in"].rearrange("(a n) -> a n", a=1))

        m_all = statp.tile([128, QB], F32)
        s_all = statp.tile([128, QB], F32)

        # ---------- phase A: emission stats (vocab-sharded logsumexp) ----------
        with (
            tc.tile_pool(name="wvp", bufs=2) as wvp,
            tc.tile_pool(name="ebvp", bufs=2) as ebvp,
            tc.tile_pool(name="scrA", bufs=1) as scrAp,
            tc.tile_pool(name="psA", bufs=4, space="PSUM") as psA,
        ):
            scrA = scrAp.tile([128, CH], F32)
            for ch in range(NCH):
                wv_t = wvp.tile([128, KT * CH], GEMM_DT, tag="wv")
                nc.sync.dma_start(
                    wv_t[:].rearrange("p (a n) -> p a n", a=KT),
                    A["wv"].rearrange("(a p) n -> p a n", p=128)[
                        :, :, ch * CH:(ch + 1) * CH
                    ],
                )
                ebv_t = ebvp.tile([1, CH], GEMM_DT, tag="ebv")
                nc.sync.dma_start(
                    ebv_t[:],
                    A["ebv"].rearrange("(a n) -> a n", a=1)[:, ch * CH:(ch + 1) * CH],
                )
                for qb in range(QB):
                    ps = psA.tile([128, CH], F32, tag="psA")
                    for kt in range(KT):
                        nc.tensor.matmul(
                            ps[:],
                            x0[:, kt * QA + qb * 128: kt * QA + (qb + 1) * 128],
                            wv_t[:, kt * CH:(kt + 1) * CH],
                            start=(kt == 0), stop=False,
                        )
                    nc.tensor.matmul(ps[:], ones[:], ebv_t[:], start=False, stop=True)
                    mq = m_all[:, qb:qb + 1]
                    sq = s_all[:, qb:qb + 1]
                    if ch == 0:
                        nc.vector.reduce_max(mq, ps[:], axis=AX.X)
                        negm = smallp.tile([128, 1], F32, tag="negm")
                        nc.vector.tensor_scalar_mul(negm[:], mq, -1.0)
                        nc.scalar.activation(
                            scrA[:], ps[:], AF.Exp, bias=negm[:], accum_out=sq,
                        )
                    else:
                        tm = smallp.tile([128, 1], F32, tag="tm")
                        nc.vector.reduce_max(tm[:], ps[:], axis=AX.X)
                        mnew = smallp.tile([128, 1], F32, tag="mnew")
                        nc.vector.tensor_max(mnew[:], mq, tm[:])
                        negm = smallp.tile([128, 1], F32, tag="negm")
                        nc.vector.tensor_scalar_mul(negm[:], mnew[:], -1.0)
                        corr = smallp.tile([128, 1], F32, tag="corr")
                        nc.scalar.activation(corr[:], mq, AF.Exp, bias=negm[:])
                        stile = smallp.tile([128, 1], F32, tag="stile")
                        nc.scalar.activation(
                            scrA[:], ps[:], AF.Exp, bias=negm[:], accum_out=stile[:],
                        )
                        nc.vector.scalar_tensor_tensor(
                            sq, sq, corr[:], stile[:], OP.mult, OP.add,
                        )
                        nc.vector.tensor_copy(mq, mnew[:])
            nc.sync.dma_start(A["m_out"].rearrange("(n p) -> p n", p=128), m_all[:])
            nc.sync.dma_start(A["s_out"].rearrange("(n p) -> p n", p=128), s_all[:])

        # ---------- phase B: grammar MLP (transposed space), q-sharded ----------
        def lin_t(w_name, b_name, func, src):
            out_t = actp.tile([128, KT * QS], F32R if GEMM_DT == F32R else F32,
                              tag="mlp_act")
            with tc.tile_pool(name="psB", bufs=4, space="PSUM") as psB:
                for sh in range(2):
                    strip = wpool.tile([128, KT * 512], GEMM_DT, tag="wstrip")
                    nc.sync.dma_start(
                        strip[:].rearrange("p (a n) -> p a n", a=KT),
                        A[w_name].rearrange("(a p) n -> p a n", p=128)[
                            :, :, sh * 512:(sh + 1) * 512
                        ],
                    )
                    for dbl in range(4):
                        db = sh * 4 + dbl
                        pp = psB.tile([128, QS], F32, tag="psB")
                        for kt in range(KT):
                            nc.tensor.matmul(
                                pp[:],
                                strip[:, kt * 512 + dbl * 128: kt * 512 + (dbl + 1) * 128],
                                src[:, kt * QS:(kt + 1) * QS],
                                start=(kt == 0), stop=(kt == KT - 1),
                            )
                        nc.scalar.activation(
                            out_t[:, db * QS:(db + 1) * QS], pp[:], func,
                            bias=biases[b_name][:, db:db + 1],
                        )
            return out_t

        h = lin_t("w_s", "b_s", AF.Identity, h0)
        t1 = lin_t("w_r11", "b_r11", AF.Relu, h)
        t2 = lin_t("w_r12", "b_r12", AF.Relu, t1)
        h2 = t2
        nc.vector.tensor_add(h2[:], t2[:], h[:])
        t3 = lin_t("w_r21", "b_r21", AF.Relu, h2)
        t4 = lin_t("w_r22", "b_r22", AF.Relu, t3)
        h3 = t4
        nc.vector.tensor_add(h3[:], t4[:], h2[:])

        # split head: two [1, 256] logit rows, log_softmax over the pair
        with tc.tile_pool(name="psS", bufs=2, space="PSUM") as psS:
            pa = psS.tile([1, QS], F32, tag="pa")
            pb = psS.tile([1, QS], F32, tag="pb")
            for kt in range(KT):
                nc.tensor.matmul(
                    pa[:], sow_t[:, kt * 2:kt * 2 + 1],
                    h3[:, kt * QS:(kt + 1) * QS],
                    start=(kt == 0), stop=(kt == KT - 1),
                )
            for kt in range(KT):
                nc.tensor.matmul(
                    pb[:], sow_t[:, kt * 2 + 1:kt * 2 + 2],
                    h3[:, kt * QS:(kt + 1) * QS],
                    start=(kt == 0), stop=(kt == KT - 1),
                )
            a_t = splitp.tile([1, QS], F32, tag="a_t")
            b_t = splitp.tile([1, QS], F32, tag="b_t")
            nc.scalar.activation(a_t[:], pa[:], AF.Identity, bias=sob_a[:])
            nc.scalar.activation(b_t[:], pb[:], AF.Identity, bias=sob_b[:])
        mx2 = splitp.tile([1, QS], F32, tag="mx2")
        nc.vector.tensor_max(mx2[:], a_t[:], b_t[:])
        da = splitp.tile([1, QS], F32, tag="da")
        db_ = splitp.tile([1, QS], F32, tag="db_")
        nc.vector.tensor_sub(da[:], a_t[:], mx2[:])
        nc.vector.tensor_sub(db_[:], b_t[:], mx2[:])
        ea = splitp.tile([1, QS], F32, tag="ea")
        eb = splitp.tile([1, QS], F32, tag="eb")
        nc.scalar.activation(ea[:], da[:], AF.Exp)
        nc.scalar.activation(eb[:], db_[:], AF.Exp)
        ssum = splitp.tile([1, QS], F32, tag="ssum")
        nc.vector.tensor_add(ssum[:], ea[:], eb[:])
        ln2 = splitp.tile([1, QS], F32, tag="ln2")
        nc.scalar.activation(ln2[:], ssum[:], AF.Ln)
        s0_t = splitp.tile([1, QS], F32, tag="s0_t")
        s1_t = splitp.tile([1, QS], F32, tag="s1_t")
        nc.vector.tensor_sub(s0_t[:], da[:], ln2[:])
        nc.vector.tensor_sub(s1_t[:], db_[:], ln2[:])
        nc.sync.dma_start(A["split_out"][0:1, :], s0_t[:])
        nc.sync.dma_start(A["split_out"][1:2, :], s1_t[:])

        # branch column: transpose spl[0, :128] -> [128, 1] via DRAM roundtrip
        nc.sync.dma_start(
            A["br_scratch"].rearrange("(a p) -> a p", a=1), s0_t[0:1, 0:128]
        )
        br_col = smallp.tile([128, 1], F32, tag="br_col")
        nc.sync.dma_start(br_col[:], A["br_scratch"].rearrange("(p a) -> p a", a=1))

        # ---------- phase C: rule log_softmax + branch -> G shards ----------
        gl_t = outp.tile([128, QR], F32, tag="gl")
        gr_t = outp.tile([128, QR], F32, tag="gr")
        mR = smallp.tile([128, 1], F32, tag="mR")
        nc.vector.reduce_max(mR[:], rwb[:], axis=AX.X)
        negmR = smallp.tile([128, 1], F32, tag="negmR")
        nc.vector.tensor_scalar_mul(negmR[:], mR[:], -1.0)
        sR1 = smallp.tile([128, 1], F32, tag="sR1")
        sR2 = smallp.tile([128, 1], F32, tag="sR2")
        # exp outputs are scratch; write them over gl_t/gr_t (overwritten below)
        nc.scalar.activation(gl_t[:], rwb[:, :QR], AF.Exp, bias=negmR[:],
                             accum_out=sR1[:])
        nc.scalar.activation(gr_t[:], rwb[:, QR:], AF.Exp, bias=negmR[:],
                             accum_out=sR2[:])
        sR = smallp.tile([128, 1], F32, tag="sR")
        nc.vector.tensor_add(sR[:], sR1[:], sR2[:])
        lnR = smallp.tile([128, 1], F32, tag="lnR")
        nc.scalar.activation(lnR[:], sR[:], AF.Ln)
        # total additive bias = branch - m - ln(s)
        tb1 = smallp.tile([128, 1], F32, tag="tb1")
        nc.vector.tensor_sub(tb1[:], br_col[:], mR[:])
        tb2 = smallp.tile([128, 1], F32, tag="tb2")
        nc.vector.tensor_sub(tb2[:], tb1[:], lnR[:])
        nc.scalar.activation(gl_t[:], rwb[:, :QR], AF.Identity, bias=tb2[:])
        nc.scalar.activation(gr_t[:], rwb[:, QR:], AF.Identity, bias=tb2[:])
        nc.sync.dma_start(A["gl_out"][:], gl_t[:])
        nc.sync.dma_start(A["gr_out"][:], gr_t[:])

        # ---------- phase D: gathered emission logits (token-sharded) ----------
        wg_t = wpool.tile([128, KT * 512], GEMM_DT, tag="wstrip")
        nc.sync.dma_start(
            wg_t[:].rearrange("p (a n) -> p a n", a=KT),
            A["wg"].rearrange("(a p) n -> p a n", p=128),
        )
        with (
            tc.tile_pool(name="psD", bufs=4, space="PSUM") as psD,
            tc.tile_pool(name="zgp", bufs=2) as zgp,
        ):
            for tb in range(TS // 128):
                for qt in range(QA // 512):
                    pp = psD.tile([128, 512], F32, tag="psD")
                    for kt in range(KT):
                        nc.tensor.matmul(
                            pp[:],
                            wg_t[:, kt * 512 + tb * 128: kt * 512 + (tb + 1) * 128],
                            x0[:, kt * QA + qt * 512: kt * QA + (qt + 1) * 512],
                            start=(kt == 0), stop=(kt == KT - 1),
                        )
                    zt = zgp.tile([128, 512], F32, tag="zg")
                    nc.scalar.activation(zt[:], pp[:], AF.Identity,
                                         bias=ebg_t[:, tb:tb + 1])
                    nc.sync.dma_start(
                        A["zg_out"][tb * 128:(tb + 1) * 128, qt * 512:(qt + 1) * 512],
                        zt[:],
                    )

        # ---------- phase E: root log_softmax ----------
        with tc.tile_pool(name="rootp", bufs=1) as rootp:
            r0_t = rootp.tile([1, QA], F32)
            nc.sync.dma_start(r0_t[:], A["r0"].rearrange("(a n) -> a n", a=1))
            p0_t = rootp.tile([1, QA], F32)
            m0 = smallp.tile([1, 1], F32, tag="m0")
            nc.vector.reduce_max(m0[:], r0_t[:], axis=AX.X)
            neg0 = smallp.tile([1, 1], F32, tag="neg0")
            nc.vector.tensor_scalar_mul(neg0[:], m0[:], -1.0)
            s0 = smallp.tile([1, 1], F32, tag="s0")
            nc.scalar.activation(p0_t[:], r0_t[:], AF.Exp, bias=neg0[:],
                                 accum_out=s0[:])
            ln0 = smallp.tile([1, 1], F32, tag="ln0")
            nc.scalar.activation(ln0[:], s0[:], AF.Ln)
            b0 = smallp.tile([1, 1], F32, tag="b0")
            nc.vector.tensor_add(b0[:], m0[:], ln0[:])
            b0n = smallp.tile([1, 1], F32, tag="b0n")
            nc.vector.tensor_scalar_mul(b0n[:], b0[:], -1.0)
            nc.scalar.activation(p0_t[:], r0_t[:], AF.Identity, bias=b0n[:])
            nc.sync.dma_start(A["p0_out"].rearrange("(a n) -> a n", a=1), p0_t[:])


_CACHED = None


def _build():
    global _CACHED
    if _CACHED is not None:
        return _CACHED
    nc = bacc.Bacc("TRN2", target_bir_lowering=False, debug=False)
    A = {}

    def din(name, shape, dt=GEMM_DT):
        A[name] = nc.dram_tensor(name, shape, dt, kind="ExternalInput").ap()

    def dout(name, shape):
        A[name] = nc.dram_tensor(name, shape, F32, kind="ExternalOutput").ap()

    din("x0T", [D, QA])
    din("wv", [D, VS])
    din("ebv", [VS])
    din("wg", [D, TS])
    din("ebg", [TS], F32)
    din("h0T", [D, QS])
    for nm in ("w_s", "w_r11", "w_r12", "w_r21", "w_r22"):
        din(nm, [D, D])
    for nm in ("b_s", "b_r11", "b_r12", "b_r21", "b_r22"):
        din(nm, [D], F32)
    din("sow", [D, 2])
    din("sob", [2], F32)
    din("rwb", [128, QA], F32)
    din("r0", [QA], F32)
    din("ones_in", [128])
    A["br_scratch"] = nc.dram_tensor("br_scratch", [128], F32).ap()

    dout("m_out", [QA])
    dout("s_out", [QA])
    dout("split_out", [2, QS])
    dout("gl_out", [128, QR])
    dout("gr_out", [128, QR])
    dout("zg_out", [TS, QA])
    dout("p0_out", [QA])

    with tile.TileContext(nc) as tc:
        _emit(tc, A)
    nc.compile()
    _CACHED = nc
    return nc


def _prepare_in_maps(inputs):
    f32 = np.float32
    nt_emb = np.asarray(inputs["nt_emb"], f32)
    emit_w = np.asarray(inputs["emit_w"], f32)
    emit_b = np.asarray(inputs["emit_b"], f32)
    rule_w = np.asarray(inputs["rule_w"], f32)
    rule_b = np.asarray(inputs["rule_b"], f32)
    x = np.asarray(inputs["x"]).reshape(-1).astype(np.int64)

    x0T = np.ascontiguousarray(nt_emb.T)
    r0 = (np.asarray(inputs["root_mask"], f32)
          + np.asarray(inputs["root_w"], f32)
          + np.asarray(inputs["root_b"], f32)).astype(f32)

    shared = {
        "ones_in": np.ones(128, np.float32),
        "x0T": x0T,
        "r0": r0,
        "sow": np.ascontiguousarray(np.asarray(inputs["sow"], f32)),
        "sob": np.asarray(inputs["sob"], f32),
    }
    wmap = {"w_s": "sw1", "w_r11": "r1w1", "w_r12": "r1w2",
            "w_r21": "r2w1", "w_r22": "r2w2"}
    bmap = {"b_s": "sb1", "b_r11": "r1b1", "b_r12": "r1b2",
            "b_r21": "r2b1", "b_r22": "r2b2"}
    for k, src in wmap.items():
        shared[k] = np.ascontiguousarray(np.asarray(inputs[src], f32))
    for k, src in bmap.items():
        shared[k] = np.asarray(inputs[src], f32)

    in_maps = []
    for c in range(NCORE):
        ids = x[c * TS:(c + 1) * TS]
        qsel = np.concatenate([
            np.arange(128 * c, 128 * (c + 1)),
            np.arange(QR + 128 * c, QR + 128 * (c + 1)),
        ])
        m = dict(shared)
        m["wv"] = np.ascontiguousarray(emit_w[:, c * VS:(c + 1) * VS])
        m["ebv"] = np.ascontiguousarray(emit_b[c * VS:(c + 1) * VS])
        m["wg"] = np.ascontiguousarray(emit_w[:, ids])
        m["ebg"] = np.ascontiguousarray(emit_b[ids])
        m["h0T"] = np.ascontiguousarray(nt_emb[qsel].T)
        m["rwb"] = np.ascontiguousarray(
            rule_w[128 * c:128 * (c + 1)] + rule_b[None, :])
        in_maps.append(m)
    return in_maps


def _postprocess(results):
    f32 = np.float32
    # global logsumexp from per-core (max, sumexp)
    m = np.stack([results[c]["m_out"] for c in range(NCORE)])   # [8, 2048]
    s = np.stack([results[c]["s_out"] for c in range(NCORE)])   # [8, 2048]
    M = m.max(axis=0)
    lse = (M + np.log((s * np.exp(m - M[None, :])).sum(axis=0))).astype(f32)

    zg = np.concatenate([results[c]["zg_out"] for c in range(NCORE)], axis=0)
    x_scores = (zg - lse[None, :]).reshape(B, S, QA).astype(f32)

    full_G_larg = np.concatenate(
        [results[c]["gl_out"] for c in range(NCORE)], axis=0)
    full_G_rarg = np.concatenate(
        [results[c]["gr_out"] for c in range(NCORE)], axis=0)

    split_scores = np.empty((QA, 2), f32)
    for c in range(NCORE):
        so = results[c]["split_out"]          # [2, 256]
        split_scores[128 * c:128 * (c + 1), 0] = so[0, :128]
        split_scores[128 * c:128 * (c + 1), 1] = so[1, :128]
        split_scores[QR + 128 * c:QR + 128 * (c + 1), 0] = so[0, 128:]
        split_scores[QR + 128 * c:QR + 128 * (c + 1), 1] = so[1, 128:]

    full_p0 = results[0]["p0_out"]
    return full_p0, full_G_larg, full_G_rarg, split_scores, x_scores


def run_on_device(inputs, **run_kwargs):
    """Build/compile (cached), run on the 8 cores, return BassKernelResults."""
    nc = _build()
    in_maps = _prepare_in_maps(inputs)
    return run_bass_kernel_spmd(nc, in_maps, core_ids=list(range(NCORE)),
                                **run_kwargs)


def kernel(**inputs):
    res = run_on_device(inputs)
    return _postprocess(res.results)
